# revision 35
# baseline (speedup 1.0000x reference)
"""Trainium2 Bass kernel for nn_Attention (sparse_attention, T=3).

Math (per batch row b, derived from the reference):
    zq = z[:, :3*2048].reshape(B, 3, D)   (q and v source)
    zk = z[:, 3*2048:].reshape(B, 3, D)
    query = zq @ wq.T + bq ; key = zk @ wk.T + bk
    scores[t,s] = query[t].key[s]/sqrt(D)
               = (zq[t] @ (wq.T @ wk) @ zk[s] + zq[t].(wq.T @ bk)
                  + (bq @ wk).zk[s] + bq.bk) / sqrt(D)
    strictly-lower entries of scores are replaced by 0 before softmax
    p = softmax(scores, axis=-1); w[s] = sum_t p[t,s]  (sum_s w[s] == 3)
    y = (sum_s w[s]*zq[s]) @ wv.T @ wo.T + 3*bv @ wo.T + 3*bo

So per core (data-parallel shard of B):
    M = wq.T @ wk (PE, natural layouts), a = wq.T @ bk, r = bq @ wk, kap = bq.bk
    G~ = zq @ M + r  (PE, needs zq tiles transposed on-chip)
    scores via DVE tensor_tensor_reduce dots + PE matvec for zq.a
    softmax (ACT exp) -> w -> zv = sum_s w_s zq_s (DVE)
    uT = wv @ zv.T + 3bv ; y = (uT.T @ wo.T) + 3bo  (PE, weights transposed
    on-chip via PE transpose)
All matmuls in bf16 (PSUM f32 accumulate); dots/softmax in f32/bf16 mix.
Verified numerically: L2 rel err ~4e-3 vs f32 reference.
"""

import sys

sys.path.insert(0, "/opt/trn_rl_repo")

import numpy as np
from concourse import bacc, bass, masks, mybir, tile
from concourse.bass_utils import run_bass_kernel_spmd

F32 = mybir.dt.float32
BF16 = mybir.dt.bfloat16
ADD = mybir.AluOpType.add
MULT = mybir.AluOpType.mult

B = 8192
D = 2048
T = 3
NCORES = 8
DC = D // 128      # 16 d-chunks
EC = D // 512      # 4 e-chunks (512-wide psum banks)
SQD = 1.0 / float(np.sqrt(np.float32(D)))


def emit(tc, aps, b_loc, stages=("p1", "p2", "p3", "p4")):
    nc = tc.nc
    z, wq, bq, wk, bk, wv, bv, wo, bo, out = (
        aps["z"], aps["wq"], aps["bq"], aps["wk"], aps["bk"],
        aps["wv"], aps["bv"], aps["wo"], aps["bo"], aps["out"],
    )
    NB = b_loc // 128

    const = tc.alloc_tile_pool(name="const", bufs=1)
    persist = tc.alloc_tile_pool(name="persist", bufs=1)

    ident = const.tile([128, 128], BF16)
    masks.make_identity(nc, ident[:])

    # --- biases ---
    # column layouts: col[p, c] = vec[c*128 + p]
    bq_col = const.tile([128, DC], F32)
    bk_col = const.tile([128, DC], F32)
    bv3_col = const.tile([128, DC], F32)
    nc.sync.dma_start(bq_col[:], bq.rearrange("(c p) -> p c", p=128))
    nc.sync.dma_start(bk_col[:], bk.rearrange("(c p) -> p c", p=128))
    nc.sync.dma_start(bv3_col[:], bv.rearrange("(c p) -> p c", p=128))
    nc.vector.tensor_scalar(bv3_col[:], bv3_col[:], 3.0, None, op0=MULT)
    bq_colbf = const.tile([128, DC], BF16)
    bk_colbf = const.tile([128, DC], BF16)
    nc.vector.tensor_copy(bq_colbf[:], bq_col[:])
    nc.vector.tensor_copy(bk_colbf[:], bk_col[:])
    bo3_row = const.tile([1, D], F32)
    nc.sync.dma_start(bo3_row[:], bo[None, :])
    nc.vector.tensor_scalar(bo3_row[:], bo3_row[:], 3.0, None, op0=MULT)

    # small persistent SBUF tensors (big ones get phase-scoped pools below)
    a_colbf = persist.tile([128, DC], BF16)        # a[d] column layout
    r_rowp = persist.tile([1, D], BF16)            # r as a single row
    ones_row = persist.tile([1, 128], BF16)        # rank-1 lhsT for r-add
    kap_col = persist.tile([128, 1], F32)          # kap/sqrt(D) per-partition

    m_pool = tc.alloc_tile_pool(name="m_pool", bufs=1)     # P1..P2
    m_bf = m_pool.tile([128, DC, D], BF16)         # M[d, e], partition = d%128

    # ---------------- Phase 1: M = wq.T @ wk, a = wq.T @ bk, r = bq @ wk ----
    with (
        tc.tile_pool(name="p1_wq", bufs=1) as p_wq,
        tc.tile_pool(name="p1_io", bufs=2) as p_io,
        tc.tile_pool(name="p1_wk", bufs=2) as p_wk,
        tc.tile_pool(name="p1_sm", bufs=2) as p_sm,
        tc.tile_pool(name="p1_psum", bufs=4, space="PSUM") as pp_m,
        tc.tile_pool(name="p1_psr", bufs=2, space="PSUM") as pp_r,
    ):
        wq_bf = p_wq.tile([128, DC, D], BF16)
        for n in range(DC):
            wq_f = p_io.tile([128, D], F32, tag="wload")
            nc.sync.dma_start(wq_f[:], wq[n * 128:(n + 1) * 128, :])
            if n % 2 == 0:
                nc.scalar.activation(wq_bf[:, n, :], wq_f[:],
                                     mybir.ActivationFunctionType.Copy)
            else:
                nc.vector.tensor_copy(wq_bf[:, n, :], wq_f[:])

        nc.vector.memset(ones_row[:], 1.0)
        for e in range(EC):
            wk_bf = p_wk.tile([128, DC, 512], BF16)
            for n in range(DC):
                wk_f = p_io.tile([128, 512], F32, tag="wkload", bufs=3)
                nc.sync.dma_start(
                    wk_f[:], wk[n * 128:(n + 1) * 128, e * 512:(e + 1) * 512])
                nc.scalar.activation(wk_bf[:, n, :], wk_f[:],
                                     mybir.ActivationFunctionType.Copy)
            for d in range(DC):
                ps = pp_m.tile([128, 512], F32)
                for n in range(DC):
                    nc.tensor.matmul(
                        ps[:], wq_bf[:, n, d * 128:(d + 1) * 128],
                        wk_bf[:, n, :], start=(n == 0), stop=(n == DC - 1))
                nc.vector.tensor_copy(m_bf[:, d, e * 512:(e + 1) * 512], ps[:])
            # r[e-slice] = bq @ wk[:, e-slice]
            ps_r = pp_r.tile([1, 512], F32)
            for n in range(DC):
                nc.tensor.matmul(ps_r[:], bq_colbf[:, n:n + 1], wk_bf[:, n, :],
                                 start=(n == 0), stop=(n == DC - 1))
            nc.vector.tensor_copy(r_rowp[:, e * 512:(e + 1) * 512], ps_r[:])

        # a = wq.T @ bk  (column layout), kap = bq.bk
        a_col = p_sm.tile([128, DC], F32, bufs=1)
        for d in range(DC):
            ps_a = pp_r.tile([128, 1], F32, tag="psa")
            for n in range(DC):
                nc.tensor.matmul(ps_a[:], wq_bf[:, n, d * 128:(d + 1) * 128],
                                 bk_colbf[:, n:n + 1],
                                 start=(n == 0), stop=(n == DC - 1))
            nc.vector.tensor_copy(a_col[:, d:d + 1], ps_a[:])
        nc.vector.tensor_copy(a_colbf[:], a_col[:])
        ps_k = pp_r.tile([1, 1], F32, tag="psa")
        for n in range(DC):
            nc.tensor.matmul(ps_k[:], bq_colbf[:, n:n + 1], bk_colbf[:, n:n + 1],
                             start=(n == 0), stop=(n == DC - 1))
        kap_row = p_sm.tile([1, 1], F32, bufs=1)
        nc.vector.tensor_copy(kap_row[:], ps_k[:])
        nc.gpsimd.partition_broadcast(kap_col[:], kap_row[:])
        nc.vector.tensor_scalar(kap_col[:], kap_col[:], SQD, None, op0=MULT)

    if not any(t.startswith("p2") for t in stages):
        m_pool.release()
        persist.release()
        const.release()
        return

    # ---------------- Phase 2: per b-tile scores/softmax/zv ----------------
    # Software-pipelined: section B (DVE/ACT dots+softmax+zv of tile ib-1)
    # is emitted before sections A/C (PE transposes + G matmuls of tile ib),
    # so the vector chain of one tile runs under the PE matmuls of the next.
    zvT_dram = nc.dram_tensor("zvT_dram", [DC, 128, b_loc], BF16).ap()
    with (
        tc.tile_pool(name="p2_io", bufs=2) as p_io,
        tc.tile_pool(name="p2_zq", bufs=1) as p_zq,
        tc.tile_pool(name="p2_g", bufs=1) as p_g,
        tc.tile_pool(name="p2_sc", bufs=1) as p_sc,
        tc.tile_pool(name="p2_psg", bufs=6, space="PSUM") as pp_g,
        tc.tile_pool(name="p2_psv", bufs=2, space="PSUM") as pp_v,
    ):
        EXP = mybir.ActivationFunctionType.Exp
        CPY = mybir.ActivationFunctionType.Copy

        def sec_a(ib):
            """loads + casts (ACT) + zq transposes for tile ib"""
            r0 = ib * 128
            st = {}
            st["zk_bf"] = p_zq.tile([128, T, D], BF16, tag="zkbf", bufs=2, name="zk_bf")
            for s in range(T):
                zk_f = p_io.tile([128, D], F32, tag="zf", bufs=3, name="zk_f")
                nc.sync.dma_start(
                    zk_f[:], z[r0:r0 + 128, (T + s) * D:(T + s + 1) * D])
                # fold the 1/sqrt(D) score scale into the k cast
                nc.scalar.activation(st["zk_bf"][:, s, :], zk_f[:], CPY,
                                     scale=SQD)
            st["zq_bf"] = p_zq.tile([128, T, D], BF16, tag="zqbf", bufs=2, name="zq_bf")
            for t in range(T):
                zq_f = p_io.tile([128, D], F32, tag="zf", bufs=3, name="zq_f")
                nc.sync.dma_start(
                    zq_f[:], z[r0:r0 + 128, t * D:(t + 1) * D])
                nc.scalar.activation(st["zq_bf"][:, t, :], zq_f[:], CPY)
            st["zqT"] = p_zq.tile([128, T, DC, 128], BF16, tag="zqT", bufs=1, name="zqT")
            for t in range(T):
                for d in range(DC):
                    nc.scalar.dma_start(st["zqT"][:, t, d, :],
                                        st["zq_bf"][:, t, d * 128:(d + 1) * 128],
                                        transpose=True)
            return st

        def sec_c(ib, st):
            """G = zq @ M + r and tvec (PE work + psum copies)"""
            gt = p_g.tile([128, T, D], BF16, tag="gt", bufs=1)
            for t in range(T):
                for e in range(EC):
                    ps = pp_g.tile([128, 512], F32)
                    # rank-1 r-add folded into the PSUM accumulation
                    nc.tensor.matmul(ps[:], ones_row[:],
                                     r_rowp[:, e * 512:(e + 1) * 512],
                                     start=True, stop=False)
                    for d in range(DC):
                        nc.tensor.matmul(
                            ps[:], st["zqT"][:, t, d, :],
                            m_bf[:, d, e * 512:(e + 1) * 512],
                            start=False, stop=(d == DC - 1))
                    nc.scalar.activation(gt[:, t, e * 512:(e + 1) * 512],
                                         ps[:], CPY)
            tvec = p_g.tile([128, T], F32, tag="tvec", bufs=1)
            for t in range(T):
                ps = pp_v.tile([128, 1], F32, tag="pstv")
                for d in range(DC):
                    nc.tensor.matmul(ps[:], st["zqT"][:, t, d, :],
                                     a_colbf[:, d:d + 1],
                                     start=(d == 0), stop=(d == DC - 1))
                nc.vector.tensor_scalar(tvec[:, t:t + 1], ps[:], SQD,
                                        kap_col[:], op0=MULT, op1=ADD)
            st["gt"] = gt
            st["tvec"] = tvec

        def sec_b(ib, st):
            """scores dots + softmax + zv for tile ib (DVE/ACT only)"""
            gt, tvec = st["gt"], st["tvec"]
            sraw = p_sc.tile([128, T, T], F32, tag="sraw", bufs=1)
            for s in range(T):
                for t in range(T):
                    scr = p_io.tile([128, D], BF16, tag="scr", bufs=2)
                    nc.vector.tensor_tensor(scr[:], gt[:, t, :],
                                            st["zk_bf"][:, s, :], op=MULT)
                    nc.vector.tensor_reduce(sraw[:, t, s:s + 1], scr[:],
                                            axis=mybir.AxisListType.X,
                                            op=ADD)
            # softmax; exp(score + tvec[t]) with masked entries = exp(0) = 1
            p_un = p_sc.tile([128, T, T], F32, tag="p_un", bufs=1)
            nc.scalar.activation(p_un[:, 0, :], sraw[:, 0, :], EXP,
                                 bias=tvec[:, 0:1])
            nc.scalar.activation(p_un[:, 1, 1:], sraw[:, 1, 1:], EXP,
                                 bias=tvec[:, 1:2])
            nc.scalar.activation(p_un[:, 2, 2:], sraw[:, 2, 2:], EXP,
                                 bias=tvec[:, 2:3])
            nc.vector.memset(p_un[:, 1, 0:1], 1.0)
            nc.vector.memset(p_un[:, 2, 0:2], 1.0)
            rsum = p_sc.tile([128, T], F32, tag="rsum", bufs=1)
            nc.vector.tensor_reduce(rsum[:], p_un[:],
                                    axis=mybir.AxisListType.X, op=ADD)
            rinv = p_sc.tile([128, T], F32, tag="rinv", bufs=1)
            nc.vector.reciprocal(rinv[:], rsum[:])
            pn = p_sc.tile([128, T, T], F32, tag="pn", bufs=1)
            for t in range(T):
                nc.vector.tensor_scalar(pn[:, t, :], p_un[:, t, :],
                                        rinv[:, t:t + 1], None, op0=MULT)
            ws = p_sc.tile([128, T], F32, tag="ws", bufs=1)
            nc.vector.tensor_reduce(ws[:], pn.rearrange("p t s -> p s t"),
                                    axis=mybir.AxisListType.X, op=ADD)
            # zv = sum_s ws[s] * zq[s]   (bf16; muls split DVE/ACT)
            zv_bf = p_sc.tile([128, D], BF16, tag="zv", bufs=2)
            zv_t1 = p_io.tile([128, D], BF16, tag="scr", bufs=2, name="zv_t1")
            nc.vector.tensor_scalar(zv_bf[:], st["zq_bf"][:, 0, :], ws[:, 0:1],
                                    None, op0=MULT)
            nc.scalar.activation(zv_t1[:], st["zq_bf"][:, 1, :], CPY,
                                 scale=ws[:, 1:2])
            nc.vector.tensor_tensor(zv_bf[:], zv_bf[:], zv_t1[:], op=ADD)
            nc.scalar.activation(zv_t1[:], st["zq_bf"][:, 2, :], CPY,
                                 scale=ws[:, 2:3])
            nc.vector.tensor_tensor(zv_bf[:], zv_bf[:], zv_t1[:], op=ADD)
            st["zv"] = zv_bf

        def sec_d(ib, st):
            """transpose zv and spill zv^T[d, b] to DRAM"""
            r0 = ib * 128
            stg = p_g.tile([128, DC, 128], BF16, tag="zvstage", bufs=2)
            for d in range(DC):
                nc.scalar.dma_start(stg[:, d, :],
                                    st["zv"][:, d * 128:(d + 1) * 128],
                                    transpose=True)
            nc.sync.dma_start(
                zvT_dram[:, :, r0:r0 + 128].rearrange("c p b -> p c b"),
                stg[:])

        state = [None] * NB
        for ib in range(NB):
            state[ib] = sec_a(ib)
            if ib > 0:
                sec_b(ib - 1, state[ib - 1])
            sec_c(ib, state[ib])
            if ib > 0:
                sec_d(ib - 1, state[ib - 1])
        sec_b(NB - 1, state[NB - 1])
        sec_d(NB - 1, state[NB - 1])

    if "p3" not in stages:
        m_pool.release()
        persist.release()
        const.release()
        return

    # ---------------- Phase 3: uT = wv @ zv.T + 3bv ------------------------
    m_pool.release()
    uT_pool = tc.alloc_tile_pool(name="uT_pool", bufs=1)    # P3..P4
    uT = uT_pool.tile([128, DC, b_loc], BF16)      # u^T[n, b]
    with (
        tc.tile_pool(name="p3_io", bufs=2) as p_io,
        tc.tile_pool(name="p3_wvT", bufs=1) as p_wvT,
        tc.tile_pool(name="p3_psu", bufs=6, space="PSUM") as pp_u,
    ):
        wvT = p_wvT.tile([128, DC, D], BF16)   # wv^T[d, n], partition = d%128
        for n in range(DC):
            wv_f = p_io.tile([128, D], F32, tag="wvf")
            nc.sync.dma_start(wv_f[:], wv[n * 128:(n + 1) * 128, :])
            wv_b = p_io.tile([128, D], BF16, tag="wvb")
            nc.scalar.activation(wv_b[:], wv_f[:],
                                 mybir.ActivationFunctionType.Copy)
            for d in range(DC):
                nc.scalar.dma_start(wvT[:, d, n * 128:(n + 1) * 128],
                                    wv_b[:, d * 128:(d + 1) * 128],
                                    transpose=True)
        bw = min(512, b_loc)
        for h in range(b_loc // bw):
            zvh = p_io.tile([128, DC, bw], BF16, tag="zvh", bufs=2)
            nc.sync.dma_start(
                zvh[:],
                zvT_dram[:, :, h * bw:(h + 1) * bw].rearrange("c p b -> p c b"))
            for n in range(DC):
                ps = pp_u.tile([128, bw], F32)
                for d in range(DC):
                    nc.tensor.matmul(
                        ps[:], wvT[:, d, n * 128:(n + 1) * 128],
                        zvh[:, d, :],
                        start=(d == 0), stop=(d == DC - 1))
                nc.vector.tensor_scalar(uT[:, n, h * bw:(h + 1) * bw], ps[:],
                                        bv3_col[:, n:n + 1], None, op0=ADD)

    if "p4" not in stages:
        uT_pool.release()
        persist.release()
        const.release()
        return

    # ---------------- Phase 4: y = uT.T @ wo.T + 3bo -----------------------
    with (
        tc.tile_pool(name="p4_io", bufs=2) as p_io,
        tc.tile_pool(name="p4_woT", bufs=1) as p_woT,
        tc.tile_pool(name="p4_y", bufs=2) as p_y,
        tc.tile_pool(name="p4_psy", bufs=6, space="PSUM") as pp_y,
    ):
        bo3_rep = p_woT.tile([128, D], F32)
        nc.gpsimd.partition_broadcast(bo3_rep[:], bo3_row[:])
        woT = p_woT.tile([128, DC, D], BF16)   # wo^T[n, g], partition = n%128
        for g in range(DC):
            wo_f = p_io.tile([128, D], F32, tag="wof")
            nc.sync.dma_start(wo_f[:], wo[g * 128:(g + 1) * 128, :])
            wo_b = p_io.tile([128, D], BF16, tag="wob")
            nc.scalar.activation(wo_b[:], wo_f[:],
                                 mybir.ActivationFunctionType.Copy)
            for n in range(DC):
                nc.scalar.dma_start(woT[:, n, g * 128:(g + 1) * 128],
                                    wo_b[:, n * 128:(n + 1) * 128],
                                    transpose=True)
        for ib in range(NB):
            y_sb = p_y.tile([128, D], F32)
            for e in range(EC):
                ps = pp_y.tile([128, 512], F32)
                for n in range(DC):
                    nc.tensor.matmul(
                        ps[:], uT[:, n, ib * 128:(ib + 1) * 128],
                        woT[:, n, e * 512:(e + 1) * 512],
                        start=(n == 0), stop=(n == DC - 1))
                nc.vector.tensor_tensor(y_sb[:, e * 512:(e + 1) * 512], ps[:],
                                        bo3_rep[:, e * 512:(e + 1) * 512],
                                        op=ADD)
            nc.sync.dma_start(out[ib * 128:(ib + 1) * 128, :], y_sb[:])

    uT_pool.release()
    persist.release()
    const.release()


def build_nc(b_loc, stages=("p1", "p2", "p3", "p4")):
    nc = bacc.Bacc("TRN2", target_bir_lowering=False, debug=False,
                   num_devices=NCORES)
    aps = {}
    aps["z"] = nc.dram_tensor("z", [b_loc, 2 * T * D], F32,
                              kind="ExternalInput").ap()
    for w in ("wq", "wk", "wv", "wo"):
        aps[w] = nc.dram_tensor(w, [D, D], F32, kind="ExternalInput").ap()
    for b_ in ("bq", "bk", "bv", "bo"):
        aps[b_] = nc.dram_tensor(b_, [D], F32, kind="ExternalInput").ap()
    aps["out"] = nc.dram_tensor("out", [b_loc, D], F32,
                                kind="ExternalOutput").ap()
    with tile.TileContext(nc) as tc:
        emit(tc, aps, b_loc, stages)
    nc.compile()
    return nc


_CACHE = {}


def _get_nc(b_loc):
    if b_loc not in _CACHE:
        _CACHE[b_loc] = build_nc(b_loc)
    return _CACHE[b_loc]


def kernel(**inputs):
    arrs = {k: np.ascontiguousarray(np.asarray(v, dtype=np.float32))
            for k, v in inputs.items()}
    b_loc = B // NCORES
    nc = _get_nc(b_loc)
    in_maps = []
    for c in range(NCORES):
        m = {k: arrs[k] for k in ("wq", "bq", "wk", "bk", "wv", "bv",
                                  "wo", "bo")}
        m["z"] = arrs["z"][c * b_loc:(c + 1) * b_loc]
        in_maps.append(m)
    res = run_bass_kernel_spmd(nc, in_maps, core_ids=list(range(NCORES)))
    return np.concatenate([r["out"] for r in res.results], axis=0)


# revision 36
# speedup vs baseline: 1.8729x; 1.8729x over previous
"""Trainium2 Bass kernel for nn_Attention (sparse_attention, T=3).

Math (per batch row b, derived from the reference):
    zq = z[:, :3*2048].reshape(B, 3, D)   (q and v source)
    zk = z[:, 3*2048:].reshape(B, 3, D)
    query = zq @ wq.T + bq ; key = zk @ wk.T + bk
    scores[t,s] = query[t].key[s]/sqrt(D)
               = (zq[t] @ (wq.T @ wk) @ zk[s] + zq[t].(wq.T @ bk)
                  + (bq @ wk).zk[s] + bq.bk) / sqrt(D)
    strictly-lower entries of scores are replaced by 0 before softmax
    p = softmax(scores, axis=-1); w[s] = sum_t p[t,s]  (sum_s w[s] == 3)
    y = (sum_s w[s]*zq[s]) @ wv.T @ wo.T + 3*bv @ wo.T + 3*bo

So per core (data-parallel shard of B):
    M = wq.T @ wk (PE, natural layouts), a = wq.T @ bk, r = bq @ wk, kap = bq.bk
    G~ = zq @ M + r  (PE, needs zq tiles transposed on-chip)
    scores via DVE tensor_tensor_reduce dots + PE matvec for zq.a
    softmax (ACT exp) -> w -> zv = sum_s w_s zq_s (DVE)
    uT = wv @ zv.T + 3bv ; y = (uT.T @ wo.T) + 3bo  (PE, weights transposed
    on-chip via PE transpose)
All matmuls in bf16 (PSUM f32 accumulate); dots/softmax in f32/bf16 mix.
Verified numerically: L2 rel err ~4e-3 vs f32 reference.
"""

import sys

sys.path.insert(0, "/opt/trn_rl_repo")

import numpy as np
from concourse import bacc, bass, masks, mybir, tile
from concourse.bass_utils import run_bass_kernel_spmd

F32 = mybir.dt.float32
BF16 = mybir.dt.bfloat16
ADD = mybir.AluOpType.add
MULT = mybir.AluOpType.mult

B = 8192
D = 2048
T = 3
NCORES = 8
DC = D // 128      # 16 d-chunks
EC = D // 512      # 4 e-chunks (512-wide psum banks)
SQD = 1.0 / float(np.sqrt(np.float32(D)))


def emit(tc, aps, b_loc, stages=("p1", "p2", "p3", "p4")):
    nc = tc.nc
    z, wq, bq, wk, bk, wv, bv, wo, bo, out = (
        aps["z"], aps["wq"], aps["bq"], aps["wk"], aps["bk"],
        aps["wv"], aps["bv"], aps["wo"], aps["bo"], aps["out"],
    )
    NB = b_loc // 128

    const = tc.alloc_tile_pool(name="const", bufs=1)
    persist = tc.alloc_tile_pool(name="persist", bufs=1)

    ident = const.tile([128, 128], BF16)
    masks.make_identity(nc, ident[:])

    # --- biases ---
    # column layouts: col[p, c] = vec[c*128 + p]
    bq_col = const.tile([128, DC], F32)
    bk_col = const.tile([128, DC], F32)
    bv3_col = const.tile([128, DC], F32)
    nc.sync.dma_start(bq_col[:], bq.rearrange("(c p) -> p c", p=128))
    nc.sync.dma_start(bk_col[:], bk.rearrange("(c p) -> p c", p=128))
    nc.sync.dma_start(bv3_col[:], bv.rearrange("(c p) -> p c", p=128))
    nc.vector.tensor_scalar(bv3_col[:], bv3_col[:], 3.0, None, op0=MULT)
    bq_colbf = const.tile([128, DC], BF16)
    bk_colbf = const.tile([128, DC], BF16)
    nc.vector.tensor_copy(bq_colbf[:], bq_col[:])
    nc.vector.tensor_copy(bk_colbf[:], bk_col[:])
    bo3_row = const.tile([1, D], F32)
    nc.sync.dma_start(bo3_row[:], bo[None, :])
    nc.vector.tensor_scalar(bo3_row[:], bo3_row[:], 3.0, None, op0=MULT)

    # small persistent SBUF tensors (big ones get phase-scoped pools below)
    a_colbf = persist.tile([128, DC], BF16)        # a[d] column layout
    r_rowp = persist.tile([1, D], BF16)            # r as a single row
    ones_row = persist.tile([1, 128], BF16)        # rank-1 lhsT for r-add
    kap_col = persist.tile([128, 1], F32)          # kap/sqrt(D) per-partition

    m_pool = tc.alloc_tile_pool(name="m_pool", bufs=1)     # P1..P2
    m_bf = m_pool.tile([128, DC, D], BF16)         # M[d, e], partition = d%128

    # ---------------- Phase 1: M = wq.T @ wk, a = wq.T @ bk, r = bq @ wk ----
    with (
        tc.tile_pool(name="p1_wq", bufs=1) as p_wq,
        tc.tile_pool(name="p1_io", bufs=2) as p_io,
        tc.tile_pool(name="p1_wk", bufs=2) as p_wk,
        tc.tile_pool(name="p1_sm", bufs=2) as p_sm,
        tc.tile_pool(name="p1_psum", bufs=4, space="PSUM") as pp_m,
        tc.tile_pool(name="p1_psr", bufs=2, space="PSUM") as pp_r,
    ):
        wq_bf = p_wq.tile([128, DC, D], BF16)
        for n in range(DC):
            wq_f = p_io.tile([128, D], F32, tag="wload")
            nc.sync.dma_start(wq_f[:], wq[n * 128:(n + 1) * 128, :])
            if n % 2 == 0:
                nc.scalar.activation(wq_bf[:, n, :], wq_f[:],
                                     mybir.ActivationFunctionType.Copy)
            else:
                nc.vector.tensor_copy(wq_bf[:, n, :], wq_f[:])

        nc.vector.memset(ones_row[:], 1.0)
        for e in range(EC):
            wk_bf = p_wk.tile([128, DC, 512], BF16)
            for n in range(DC):
                wk_f = p_io.tile([128, 512], F32, tag="wkload", bufs=3)
                nc.sync.dma_start(
                    wk_f[:], wk[n * 128:(n + 1) * 128, e * 512:(e + 1) * 512])
                nc.scalar.activation(wk_bf[:, n, :], wk_f[:],
                                     mybir.ActivationFunctionType.Copy)
            for d in range(DC):
                ps = pp_m.tile([128, 512], F32)
                for n in range(DC):
                    nc.tensor.matmul(
                        ps[:], wq_bf[:, n, d * 128:(d + 1) * 128],
                        wk_bf[:, n, :], start=(n == 0), stop=(n == DC - 1))
                nc.vector.tensor_copy(m_bf[:, d, e * 512:(e + 1) * 512], ps[:])
            # r[e-slice] = bq @ wk[:, e-slice]
            ps_r = pp_r.tile([1, 512], F32)
            for n in range(DC):
                nc.tensor.matmul(ps_r[:], bq_colbf[:, n:n + 1], wk_bf[:, n, :],
                                 start=(n == 0), stop=(n == DC - 1))
            nc.vector.tensor_copy(r_rowp[:, e * 512:(e + 1) * 512], ps_r[:])

        # a = wq.T @ bk  (column layout), kap = bq.bk
        a_col = p_sm.tile([128, DC], F32, bufs=1)
        for d in range(DC):
            ps_a = pp_r.tile([128, 1], F32, tag="psa")
            for n in range(DC):
                nc.tensor.matmul(ps_a[:], wq_bf[:, n, d * 128:(d + 1) * 128],
                                 bk_colbf[:, n:n + 1],
                                 start=(n == 0), stop=(n == DC - 1))
            nc.vector.tensor_copy(a_col[:, d:d + 1], ps_a[:])
        nc.vector.tensor_copy(a_colbf[:], a_col[:])
        ps_k = pp_r.tile([1, 1], F32, tag="psa")
        for n in range(DC):
            nc.tensor.matmul(ps_k[:], bq_colbf[:, n:n + 1], bk_colbf[:, n:n + 1],
                             start=(n == 0), stop=(n == DC - 1))
        kap_row = p_sm.tile([1, 1], F32, bufs=1)
        nc.vector.tensor_copy(kap_row[:], ps_k[:])
        nc.gpsimd.partition_broadcast(kap_col[:], kap_row[:])
        nc.vector.tensor_scalar(kap_col[:], kap_col[:], SQD, None, op0=MULT)

    if not any(t.startswith("p2") for t in stages):
        m_pool.release()
        persist.release()
        const.release()
        return

    # ---------------- Phase 2: per b-tile scores/softmax/zv ----------------
    # Software-pipelined: section B (DVE/ACT dots+softmax+zv of tile ib-1)
    # is emitted before sections A/C (PE transposes + G matmuls of tile ib),
    # so the vector chain of one tile runs under the PE matmuls of the next.
    zvT_dram = nc.dram_tensor("zvT_dram", [DC, 128, b_loc], BF16).ap()
    with (
        tc.tile_pool(name="p2_io", bufs=2) as p_io,
        tc.tile_pool(name="p2_zq", bufs=1) as p_zq,
        tc.tile_pool(name="p2_g", bufs=1) as p_g,
        tc.tile_pool(name="p2_sc", bufs=1) as p_sc,
        tc.tile_pool(name="p2_psg", bufs=6, space="PSUM") as pp_g,
        tc.tile_pool(name="p2_psv", bufs=2, space="PSUM") as pp_v,
    ):
        EXP = mybir.ActivationFunctionType.Exp
        CPY = mybir.ActivationFunctionType.Copy

        def sec_a(ib):
            """loads + casts (ACT) + zq transposes for tile ib"""
            r0 = ib * 128
            st = {}
            st["zk_bf"] = p_zq.tile([128, T, D], BF16, tag="zkbf", bufs=2, name="zk_bf")
            for s in range(T):
                zk_f = p_io.tile([128, D], F32, tag="zf", bufs=3, name="zk_f")
                nc.sync.dma_start(
                    zk_f[:], z[r0:r0 + 128, (T + s) * D:(T + s + 1) * D])
                # fold the 1/sqrt(D) score scale into the k cast
                nc.scalar.activation(st["zk_bf"][:, s, :], zk_f[:], CPY,
                                     scale=SQD)
            st["zq_bf"] = p_zq.tile([128, T, D], BF16, tag="zqbf", bufs=2, name="zq_bf")
            for t in range(T):
                zq_f = p_io.tile([128, D], F32, tag="zf", bufs=3, name="zq_f")
                nc.sync.dma_start(
                    zq_f[:], z[r0:r0 + 128, t * D:(t + 1) * D])
                nc.scalar.activation(st["zq_bf"][:, t, :], zq_f[:], CPY)
            st["zqT"] = p_zq.tile([128, T, DC, 128], BF16, tag="zqT", bufs=1, name="zqT")
            for t in range(T):
                nc.scalar.dma_start_transpose(st["zqT"][:, t, :, :],
                                              st["zq_bf"][:, t, :])
            return st

        def sec_c(ib, st):
            """G = zq @ M + r and tvec (PE work + psum copies)"""
            gt = p_g.tile([128, T, D], BF16, tag="gt", bufs=1)
            for t in range(T):
                for e in range(EC):
                    ps = pp_g.tile([128, 512], F32)
                    # rank-1 r-add folded into the PSUM accumulation
                    nc.tensor.matmul(ps[:], ones_row[:],
                                     r_rowp[:, e * 512:(e + 1) * 512],
                                     start=True, stop=False)
                    for d in range(DC):
                        nc.tensor.matmul(
                            ps[:], st["zqT"][:, t, d, :],
                            m_bf[:, d, e * 512:(e + 1) * 512],
                            start=False, stop=(d == DC - 1))
                    nc.scalar.activation(gt[:, t, e * 512:(e + 1) * 512],
                                         ps[:], CPY)
            tvec = p_g.tile([128, T], F32, tag="tvec", bufs=1)
            for t in range(T):
                ps = pp_v.tile([128, 1], F32, tag="pstv")
                for d in range(DC):
                    nc.tensor.matmul(ps[:], st["zqT"][:, t, d, :],
                                     a_colbf[:, d:d + 1],
                                     start=(d == 0), stop=(d == DC - 1))
                nc.vector.tensor_scalar(tvec[:, t:t + 1], ps[:], SQD,
                                        kap_col[:], op0=MULT, op1=ADD)
            st["gt"] = gt
            st["tvec"] = tvec

        def sec_b(ib, st):
            """scores dots + softmax + zv for tile ib (DVE/ACT only)"""
            gt, tvec = st["gt"], st["tvec"]
            sraw = p_sc.tile([128, T, T], F32, tag="sraw", bufs=1)
            for s in range(T):
                for t in range(T):
                    scr = p_io.tile([128, D], BF16, tag="scr", bufs=2)
                    nc.vector.tensor_tensor(scr[:], gt[:, t, :],
                                            st["zk_bf"][:, s, :], op=MULT)
                    nc.vector.tensor_reduce(sraw[:, t, s:s + 1], scr[:],
                                            axis=mybir.AxisListType.X,
                                            op=ADD)
            # softmax; exp(score + tvec[t]) with masked entries = exp(0) = 1
            p_un = p_sc.tile([128, T, T], F32, tag="p_un", bufs=1)
            nc.scalar.activation(p_un[:, 0, :], sraw[:, 0, :], EXP,
                                 bias=tvec[:, 0:1])
            nc.scalar.activation(p_un[:, 1, 1:], sraw[:, 1, 1:], EXP,
                                 bias=tvec[:, 1:2])
            nc.scalar.activation(p_un[:, 2, 2:], sraw[:, 2, 2:], EXP,
                                 bias=tvec[:, 2:3])
            nc.vector.memset(p_un[:, 1, 0:1], 1.0)
            nc.vector.memset(p_un[:, 2, 0:2], 1.0)
            rsum = p_sc.tile([128, T], F32, tag="rsum", bufs=1)
            nc.vector.tensor_reduce(rsum[:], p_un[:],
                                    axis=mybir.AxisListType.X, op=ADD)
            rinv = p_sc.tile([128, T], F32, tag="rinv", bufs=1)
            nc.vector.reciprocal(rinv[:], rsum[:])
            pn = p_sc.tile([128, T, T], F32, tag="pn", bufs=1)
            for t in range(T):
                nc.vector.tensor_scalar(pn[:, t, :], p_un[:, t, :],
                                        rinv[:, t:t + 1], None, op0=MULT)
            ws = p_sc.tile([128, T], F32, tag="ws", bufs=1)
            nc.vector.tensor_reduce(ws[:], pn.rearrange("p t s -> p s t"),
                                    axis=mybir.AxisListType.X, op=ADD)
            # zv = sum_s ws[s] * zq[s]   (bf16; muls split DVE/ACT)
            zv_bf = p_sc.tile([128, D], BF16, tag="zv", bufs=2)
            zv_t1 = p_io.tile([128, D], BF16, tag="scr", bufs=2, name="zv_t1")
            nc.vector.tensor_scalar(zv_bf[:], st["zq_bf"][:, 0, :], ws[:, 0:1],
                                    None, op0=MULT)
            nc.scalar.activation(zv_t1[:], st["zq_bf"][:, 1, :], CPY,
                                 scale=ws[:, 1:2])
            nc.vector.tensor_tensor(zv_bf[:], zv_bf[:], zv_t1[:], op=ADD)
            nc.scalar.activation(zv_t1[:], st["zq_bf"][:, 2, :], CPY,
                                 scale=ws[:, 2:3])
            nc.vector.tensor_tensor(zv_bf[:], zv_bf[:], zv_t1[:], op=ADD)
            st["zv"] = zv_bf

        def sec_d(ib, st):
            """transpose zv and spill zv^T[d, b] to DRAM"""
            r0 = ib * 128
            stg = p_g.tile([128, DC, 128], BF16, tag="zvstage", bufs=2)
            nc.scalar.dma_start_transpose(stg[:], st["zv"][:])
            nc.sync.dma_start(
                zvT_dram[:, :, r0:r0 + 128].rearrange("c p b -> p c b"),
                stg[:])

        state = [None] * NB
        for ib in range(NB):
            state[ib] = sec_a(ib)
            if ib > 0:
                sec_b(ib - 1, state[ib - 1])
            sec_c(ib, state[ib])
            if ib > 0:
                sec_d(ib - 1, state[ib - 1])
        sec_b(NB - 1, state[NB - 1])
        sec_d(NB - 1, state[NB - 1])

    if "p3" not in stages:
        m_pool.release()
        persist.release()
        const.release()
        return

    # ---------------- Phase 3: uT = wv @ zv.T + 3bv ------------------------
    m_pool.release()
    uT_pool = tc.alloc_tile_pool(name="uT_pool", bufs=1)    # P3..P4
    uT = uT_pool.tile([128, DC, b_loc], BF16)      # u^T[n, b]
    with (
        tc.tile_pool(name="p3_io", bufs=2) as p_io,
        tc.tile_pool(name="p3_wvT", bufs=1) as p_wvT,
        tc.tile_pool(name="p3_psu", bufs=6, space="PSUM") as pp_u,
    ):
        wvT = p_wvT.tile([128, DC, D], BF16)   # wv^T[d, n], partition = d%128
        for n in range(DC):
            wv_f = p_io.tile([128, D], F32, tag="wvf")
            nc.sync.dma_start(wv_f[:], wv[n * 128:(n + 1) * 128, :])
            wv_b = p_io.tile([128, D], BF16, tag="wvb")
            nc.scalar.activation(wv_b[:], wv_f[:],
                                 mybir.ActivationFunctionType.Copy)
            nc.scalar.dma_start_transpose(wvT[:, :, n * 128:(n + 1) * 128],
                                          wv_b[:])
        bw = min(512, b_loc)
        for h in range(b_loc // bw):
            zvh = p_io.tile([128, DC, bw], BF16, tag="zvh", bufs=2)
            nc.sync.dma_start(
                zvh[:],
                zvT_dram[:, :, h * bw:(h + 1) * bw].rearrange("c p b -> p c b"))
            for n in range(DC):
                ps = pp_u.tile([128, bw], F32)
                for d in range(DC):
                    nc.tensor.matmul(
                        ps[:], wvT[:, d, n * 128:(n + 1) * 128],
                        zvh[:, d, :],
                        start=(d == 0), stop=(d == DC - 1))
                nc.vector.tensor_scalar(uT[:, n, h * bw:(h + 1) * bw], ps[:],
                                        bv3_col[:, n:n + 1], None, op0=ADD)

    if "p4" not in stages:
        uT_pool.release()
        persist.release()
        const.release()
        return

    # ---------------- Phase 4: y = uT.T @ wo.T + 3bo -----------------------
    with (
        tc.tile_pool(name="p4_io", bufs=2) as p_io,
        tc.tile_pool(name="p4_woT", bufs=1) as p_woT,
        tc.tile_pool(name="p4_y", bufs=2) as p_y,
        tc.tile_pool(name="p4_psy", bufs=6, space="PSUM") as pp_y,
    ):
        bo3_rep = p_woT.tile([128, D], F32)
        nc.gpsimd.partition_broadcast(bo3_rep[:], bo3_row[:])
        woT = p_woT.tile([128, DC, D], BF16)   # wo^T[n, g], partition = n%128
        for g in range(DC):
            wo_f = p_io.tile([128, D], F32, tag="wof")
            nc.sync.dma_start(wo_f[:], wo[g * 128:(g + 1) * 128, :])
            wo_b = p_io.tile([128, D], BF16, tag="wob")
            nc.scalar.activation(wo_b[:], wo_f[:],
                                 mybir.ActivationFunctionType.Copy)
            nc.scalar.dma_start_transpose(woT[:, :, g * 128:(g + 1) * 128],
                                          wo_b[:])
        for ib in range(NB):
            y_sb = p_y.tile([128, D], F32)
            for e in range(EC):
                ps = pp_y.tile([128, 512], F32)
                for n in range(DC):
                    nc.tensor.matmul(
                        ps[:], uT[:, n, ib * 128:(ib + 1) * 128],
                        woT[:, n, e * 512:(e + 1) * 512],
                        start=(n == 0), stop=(n == DC - 1))
                nc.vector.tensor_tensor(y_sb[:, e * 512:(e + 1) * 512], ps[:],
                                        bo3_rep[:, e * 512:(e + 1) * 512],
                                        op=ADD)
            nc.sync.dma_start(out[ib * 128:(ib + 1) * 128, :], y_sb[:])

    uT_pool.release()
    persist.release()
    const.release()


def build_nc(b_loc, stages=("p1", "p2", "p3", "p4")):
    nc = bacc.Bacc("TRN2", target_bir_lowering=False, debug=False,
                   num_devices=NCORES)
    aps = {}
    aps["z"] = nc.dram_tensor("z", [b_loc, 2 * T * D], F32,
                              kind="ExternalInput").ap()
    for w in ("wq", "wk", "wv", "wo"):
        aps[w] = nc.dram_tensor(w, [D, D], F32, kind="ExternalInput").ap()
    for b_ in ("bq", "bk", "bv", "bo"):
        aps[b_] = nc.dram_tensor(b_, [D], F32, kind="ExternalInput").ap()
    aps["out"] = nc.dram_tensor("out", [b_loc, D], F32,
                                kind="ExternalOutput").ap()
    with tile.TileContext(nc) as tc:
        emit(tc, aps, b_loc, stages)
    nc.compile()
    return nc


_CACHE = {}


def _get_nc(b_loc):
    if b_loc not in _CACHE:
        _CACHE[b_loc] = build_nc(b_loc)
    return _CACHE[b_loc]


def kernel(**inputs):
    arrs = {k: np.ascontiguousarray(np.asarray(v, dtype=np.float32))
            for k, v in inputs.items()}
    b_loc = B // NCORES
    nc = _get_nc(b_loc)
    in_maps = []
    for c in range(NCORES):
        m = {k: arrs[k] for k in ("wq", "bq", "wk", "bk", "wv", "bv",
                                  "wo", "bo")}
        m["z"] = arrs["z"][c * b_loc:(c + 1) * b_loc]
        in_maps.append(m)
    res = run_bass_kernel_spmd(nc, in_maps, core_ids=list(range(NCORES)))
    return np.concatenate([r["out"] for r in res.results], axis=0)
